# revision 25
# baseline (speedup 1.0000x reference)
"""DigitalMapper kernel for 8 trn2 NeuronCores.

Math: reference computes  out = (x @ softmax(W, axis=1).T) > 0.5  with
x in {0,1}.  Let E = exp(W) (row-unnormalized).  Then

  out[b,o] > 0.5  <=>  sum_i (2*x[b,i] - 1) * E[o,i] > 0

so the softmax divide, the row-max subtraction and the threshold fold
into a zero-test on a +-1 matmul (|W| <= 5.5 so exp(W) <= 185 fits
fp8e4's 240 max).

All-fp8 DoubleRow pipeline (0.5 PE-cycles/contraction-row): E is split
on-device into a 2-term fp8e4 cascade

  Q0 = e4m3(E);  Q1 = e4m3(E - Q0)

giving ~10.3 effective mantissa bits.  Measured 1640/8.4M output
bit-flips vs the f64 reference = rel 1.98e-2, under the 2e-2 gate
(NQ2 extra third-term k-pairs are available as margin fallback).
x is +-1, exact in fp8e4; both passes accumulate into one PSUM bank:

  S = xb @ (Q0 + Q1).T   (fp32 PSUM),  out = S > 0.

Per-kp prep chain (engine-balanced, all rates under the 1.71us/kp PE
slot rate): ACT exp->f32 ef (1.04us); DVE cast ef->fp8 eh (0.59us,
2x_2p mode); Pool sub (ef - eh)->fp8 q1 (0.85us).

Sharding: 2 batch-groups x 4 out-feature groups.  Each core gets
x.T[:, bg*2048:...] host-encoded as fp8e4 +-1 bytes (bijective
re-encoding of the 0/1 input) and W.T[:, og*512:...] as round(W*4096)
int16 (halves the weight DMA; the 2^-13 quantization is far below the
fp8-cascade residual), and produces a [2048, 512] block.

Schedule: wave-1 = m-tiles 0..7 kp-slotted against the input stream
(slot k: q1[k-1] then hi[k] passes), tail closes m0..7 with evictions
alternating DVE is_gt / ACT Sign (Sign table pre-warmed after the last
exp; host maps the +-1 code back to {0,1}).  Wave-2 = m8..15, all
planes resident: kp-major 0..4, then an m-sweep over kp 5..7 that
staggers the closes so evictions pipeline behind the PE instead of
bursting at the end.  Output DMAs are batched [4,4,4,2,1,1] m-tiles
to cut SP descriptor time; the last groups stay small for tail latency.
"""

import sys

sys.path.insert(0, "/opt/trn_rl_repo")

import numpy as np

BATCH, IN_F, OUT_F = 4096, 2048, 2048
N_CORES = 8
BG, OG = 2, 4  # batch groups x out-feature groups
B_PER = BATCH // BG  # 2048 batch rows per core
O_PER = OUT_F // OG  # 512 out features per core
P = 128
KT = IN_F // P  # 16 contraction tiles
KP = KT // 2  # 8 DoubleRow k-pairs
MT = B_PER // P  # 16 output row tiles per core
W1 = 8  # wave-1 m-tiles (= PSUM banks)
NQ2 = 0  # k-pairs carrying a 3rd (q2) residual pass; 0 measures
         # 1640/8.4M flips (rel 1.98e-2) - under the 2e-2 gate.
         # Bump to 1 (rel 1.92e-2) if the hw flip count drifts up.
SWEEP_KP = 5  # wave-2: kp >= SWEEP_KP run m-major with inline evicts

_COMPILED = {}


def _patch_tile_drain():
    """walrus in this container allows only ONE sem-wait per CTRL (Drain/NOP)
    instruction; Tile's kernel-tail drain aggregates one wait per live
    semaphore.  Split the waits across a chain of SP nops."""
    import concourse.mybir as mybir
    import concourse.tile as tile_mod
    from concourse.vector_clock import ScopedClock

    if getattr(tile_mod.TileContext, "_drain_split_patched", False):
        return

    def _drain_and_barrier_split(self, tick_clock, wait_clock):
        nc = self.nc
        drain_inst = nc.sync.drain()
        wait_clock.add_sem_waits(
            drain_inst.ins, ScopedClock({None: tick_clock.global_clock})
        )
        si = drain_inst.ins.sync_info
        waits = list(si.on_wait) if si is not None else []
        if len(waits) > 1:
            si.on_wait.clear()
            si.on_wait.extend(waits[:1])
            for w in waits[1:]:
                nop = nc.sync.nop(nofuse=True)
                if nop.ins.sync_info is None:
                    nop.ins.sync_info = mybir.SyncInfo(on_wait=[], on_update=[])
                nop.ins.sync_info.on_wait.append(w)
        nc.all_engine_barrier()
        assert self.sems is not None
        popped = nc._tile_sem_poison_stack.pop()
        assert popped is self._sem_poison
        nc.clear_and_free_semaphores(list(self.sems.allocated().values()))
        nc.all_engine_barrier()

    tile_mod.TileContext._drain_and_barrier = _drain_and_barrier_split
    tile_mod.TileContext._drain_split_patched = True


def _split_multi_waits(nc):
    """walrus here allows very few sem-waits per instruction.  Hoist extra
    waits onto same-engine NOPs placed immediately before the instruction
    (same blocking point, engine executes in order).  DMA-queue instructions
    keep their waits - their sync runs through the DGE queues."""
    import concourse.mybir as mybir

    n = 0
    for f in nc.m.functions:
        for bb in f.blocks:
            new_insts = []
            for inst in bb.instructions:
                si = inst.sync_info
                if si is not None and si.on_wait and len(si.on_wait) > 1:
                    waits = list(si.on_wait)
                    si.on_wait.clear()
                    si.on_wait.append(waits[0])
                    for w in waits[1:]:
                        n += 1
                        new_insts.append(
                            mybir.InstNoOp(
                                name=f"wsplit-{n}",
                                opcode="NoOp",
                                engine=inst.engine,
                                sync_info=mybir.SyncInfo(on_wait=[w], on_update=[]),
                                bass_nofuse=True,
                            )
                        )
                new_insts.append(inst)
            if n:
                try:
                    bb.instructions[:] = new_insts
                except TypeError:
                    bb.instructions = new_insts
    return n


def _build(split_waits: bool = True):
    """One core's SPMD program (see module docstring)."""
    import concourse.bass as bass
    import concourse.library_config as library_config
    import concourse.mybir as mybir
    import concourse.tile as tile

    _patch_tile_drain()

    f32 = mybir.dt.float32
    f8 = mybir.dt.float8e4
    u8 = mybir.dt.uint8
    i16 = mybir.dt.int16
    Alu = mybir.AluOpType
    Act = mybir.ActivationFunctionType
    DR = mybir.MatmulPerfMode.DoubleRow

    _build.mm_labels = {}
    # The last three output groups go out via prepared kv_writeback
    # descriptors fired by one trigger_dma - the triggered path skips the
    # HWDGE seq+gen+delay chain (~1.7us off the kernel tail)
    nc = bass.Bass()
    # xt holds host-encoded +-1 in fp8e4 bytes, [IN_F, B_PER]
    xt = nc.dram_tensor("xt", [IN_F, B_PER], f8, kind="ExternalInput")
    # W arrives as round(W*4096) int16: halves the weight DMA; the 2^-13
    # quantization is far below the fp8-cascade residual
    wt = nc.dram_tensor("wt", [IN_F, O_PER], i16, kind="ExternalInput")
    out = nc.dram_tensor("out", [B_PER, O_PER], u8, kind="ExternalOutput")

    # out-DMA m-tile groups: big early groups amortize the 565ns SP
    # descriptor time; the tail groups stay small for end latency.
    # Groups >= TRIG_G0 use the prepared/triggered SWDGE path.
    OGROUPS = [(0, 4), (4, 8), (8, 12), (12, 14), (14, 15), (15, 16)]
    TRIG_G0 = 3

    with tile.TileContext(nc) as tc:
        with (
            tc.tile_pool(name="xq", bufs=1) as xq_pool,
            tc.tile_pool(name="wr", bufs=3) as wr_pool,
            tc.tile_pool(name="ef", bufs=3) as ef_pool,
            tc.tile_pool(name="eh", bufs=1) as eh_pool,
            tc.tile_pool(name="q1", bufs=1) as q1_pool,
            tc.tile_pool(name="qx", bufs=1) as qx_pool,
            tc.tile_pool(name="ps", bufs=1, space="PSUM") as ps_pool,
            tc.tile_pool(name="ot", bufs=1) as ot_pool,
        ):
            # gpsimd 'proxy' library carries both tensor_tensor (the q1
            # subs) and kv_writeback (the triggered output stores)
            nc.gpsimd.load_library(library_config.proxy)
            # touch Exp immediately so the ACT table load overlaps the
            # first input DMAs instead of the first exp
            warm = ef_pool.tile([P, 1], f32, name="warm", tag="warm")
            nc.vector.memset(warm[:], 0.0)
            nc.scalar.activation(warm[:], warm[:], Act.Exp)
            # PE p-state warm-up: the clock reaches 2.4GHz only after 3us of
            # CONTINUOUS PE activity (pe_busy_start resets when PE idles), so
            # a bridge of dummy matmuls must keep PE busy from t~0.6 until
            # the first real matmul (~4.1us) or the early real mms pay 2x.
            wmm = ef_pool.tile([P, 2, P], f8, name="wmm", tag="wmm")
            nc.vector.memset(wmm[:], 0.0)
            wm2 = ef_pool.tile([P, 2, O_PER], f8, name="wm2", tag="wm2")
            nc.vector.memset(wm2[:], 0.0)

            xq, eh, q1, q2 = {}, {}, {}, {}

            def issue_x(kp, part):
                # part "a": batch cols 0:1024 (wave-1 m-tiles);
                # part "b": cols 1024:2048 (wave-2), streamed later
                h = B_PER // 2
                if part == "a":
                    t = xq_pool.tile(
                        [P, 2, B_PER], f8, name=f"xq{kp}", tag=f"xq{kp}"
                    )
                    xq[kp] = t
                t = xq[kp]
                src = xt[kp * 2 * P : (kp + 1) * 2 * P, :].rearrange(
                    "(j p) b -> p j b", j=2
                )
                if part == "a":
                    nc.sync.dma_start(t[:, :, :h], src[:, :, :h])
                else:
                    nc.sync.dma_start(t[:, :, h:], src[:, :, h:])

            def issue_w_and_prep(kp):
                """DMA w[kp], then: ACT exp->f32 ef; DVE cast->fp8 eh;
                Pool sub (ef-eh)->fp8 q1.  kp<NQ2 additionally produce a
                second residual plane q2 = e4m3(d1 - q1) via an f32 d1."""
                wr = wr_pool.tile([P, 2, O_PER], i16, name="wr", tag="wr")
                src = wt[kp * 2 * P : (kp + 1) * 2 * P, :].rearrange(
                    "(j p) n -> p j n", j=2
                )
                ehk = eh_pool.tile([P, 2, O_PER], f8, name=f"eh{kp}", tag=f"eh{kp}")
                q1k = q1_pool.tile([P, 2, O_PER], f8, name=f"q1{kp}", tag=f"q1{kp}")
                ef = ef_pool.tile([P, 2, O_PER], f32, name="ef", tag="ef")
                # geometric ramp on the first k-pairs so the first eh/q1
                # planes land ~1us earlier (prep in col-chunks; the DMA
                # stays in >=256-col pieces - smaller pieces pay the
                # sub-512B descriptor penalty and gain nothing on the bus)
                if kp == 0:
                    dma_bounds = [0, 256, O_PER]
                    prep_bounds = [0, 128, 256, 384, O_PER]
                elif kp == 1:
                    dma_bounds = [0, O_PER]
                    prep_bounds = [0, 256, O_PER]
                else:
                    dma_bounds = [0, O_PER]
                    prep_bounds = [0, O_PER]
                for c in range(len(dma_bounds) - 1):
                    sl = slice(dma_bounds[c], dma_bounds[c + 1])
                    nc.sync.dma_start(wr[:, :, sl], src[:, :, sl])
                for c in range(len(prep_bounds) - 1):
                    sl = slice(prep_bounds[c], prep_bounds[c + 1])
                    nc.scalar.activation(
                        ef[:, :, sl], wr[:, :, sl], Act.Exp, scale=2.0**-12
                    )
                    nc.vector.tensor_copy(ehk[:, :, sl], ef[:, :, sl])
                    if kp < NQ2:
                        d1 = qx_pool.tile(
                            [P, 2, O_PER], f32, name=f"d1{kp}", tag=f"d1{kp}"
                        )
                        q2k = qx_pool.tile(
                            [P, 2, O_PER], f8, name=f"q2{kp}", tag=f"q2{kp}"
                        )
                        nc.gpsimd.tensor_tensor(
                            d1[:, :, sl], ef[:, :, sl], ehk[:, :, sl], Alu.subtract
                        )
                        nc.vector.tensor_copy(q1k[:, :, sl], d1[:, :, sl])
                        nc.gpsimd.tensor_tensor(
                            q2k[:, :, sl], d1[:, :, sl], q1k[:, :, sl], Alu.subtract
                        )
                        q2[kp] = q2k
                    else:
                        nc.gpsimd.tensor_tensor(
                            q1k[:, :, sl], ef[:, :, sl], ehk[:, :, sl], Alu.subtract
                        )
                eh[kp] = ehk
                q1[kp] = q1k

            def psum_tile(m):
                return ps_pool.tile(
                    [P, O_PER], f32, name=f"pn_{m % W1}", tag=f"pn_{m % W1}"
                )

            pn = {m: psum_tile(m) for m in range(W1)}

            # ramp the PE clock while the first inputs stream in; the real
            # hi(0) pass overwrites this bank via start=True.  Small mms
            # (on the quickly-memset wmm) start the ramp at ~0.4us and
            # bridge until the big wm2 tile is set; big mms then keep PE
            # busy to ~4.2us so the ramp never resets before real work.
            N_WARM_SMALL, N_WARM_BIG = 16, 15
            for i in range(N_WARM_SMALL):
                nc.tensor.matmul(pn[0][:, :P], wmm[:], wmm[:],
                                 start=(i == 0), stop=(i == N_WARM_SMALL - 1),
                                 perf_mode=DR)
            for i in range(N_WARM_BIG):
                nc.tensor.matmul(pn[1][:], wmm[:], wm2[:], start=(i == 0),
                                 stop=(i == N_WARM_BIG - 1), perf_mode=DR)

            def mm(ps, pass_planes, kp, m, start=False, stop=False,
                   nsl=slice(0, O_PER)):
                label = ('hi' if pass_planes is eh else 'q1' if pass_planes is q1 else 'q2', kp, m)
                r = nc.tensor.matmul(
                    ps[:, nsl],
                    xq[kp][:, :, m * P : (m + 1) * P],
                    pass_planes[kp][:, :, nsl],
                    start=start,
                    stop=stop,
                    perf_mode=DR,
                )
                _build.mm_labels[r.ins.name] = label

            ot_tiles = {}
            for gi, (m0, m1) in enumerate(OGROUPS):
                ot_tiles[gi] = ot_pool.tile(
                    [P, m1 - m0, O_PER], u8, name=f"ot{gi}", tag=f"ot{gi}"
                )
            # zero ctx indices for the kv_writeback preps (shared, int32)
            kvidx = ot_pool.tile([P, 4], mybir.dt.int32, name="kvidx", tag="kvidx")
            nc.vector.memset(kvidx[:], 0)
            dma_sem = nc.alloc_semaphore("swdge_out_dma")

            def group_of(m):
                for gi, (m0, m1) in enumerate(OGROUPS):
                    if m0 <= m < m1:
                        return gi, m - m0
                raise AssertionError(m)

            def evict(ps, m, eng_act, nsl=slice(0, O_PER)):
                # threshold S>0.  DVE: is_gt -> {0,1}.  ACT: Sign -> {-1,0,1}
                # (host maps code 1 -> 1.0, everything else -> 0.0).
                gi, mi = group_of(m)
                dst = ot_tiles[gi][:, mi, nsl]
                if eng_act:
                    nc.scalar.activation(dst, ps[:, nsl], Act.Sign)
                else:
                    nc.vector.tensor_scalar(dst, ps[:, nsl], 0.0, None, Alu.is_gt)

            def group_dma(gi):
                m0, m1 = OGROUPS[gi]
                if gi >= TRIG_G0:
                    return  # handled by the prepared/triggered path
                dst = out[m0 * P : m1 * P, :].rearrange(
                    "(j p) n -> p j n", j=m1 - m0
                )
                nc.sync.dma_start(dst, ot_tiles[gi][:])

            def prep_and_trigger_tail():
                # kv_writeback with idx 0 is a plain paged store:
                # out[b, p, 0, :] = ot[p, 0, b, :] for the group's m-tiles.
                # Emitted AFTER the evicts so Tile defers the RAW edges to
                # the trigger; the preps themselves have no sync waits and
                # execute early on the idle Pool engine.
                for gi in range(TRIG_G0, len(OGROUPS)):
                    m0, m1 = OGROUPS[gi]
                    g = m1 - m0
                    dst = out[m0 * P : m1 * P, :].rearrange(
                        "(b p) (o n) -> b p o n", b=g, o=1
                    )
                    src = ot_tiles[gi][:].rearrange("p (o b) n -> p o b n", o=1)
                    nc.gpsimd.kv_writeback(
                        dst,
                        src,
                        kvidx[:, :g],
                        prepare_only=True,
                        sem=dma_sem,
                    )
                nc.gpsimd.trigger_dma(count=None)

            # ---- wave 1: kp-slotted while inputs stream --------------------
            # DMA issue order: w one slot ahead of xa so the exp/cast chain
            # hides under the x transfer; xb (wave-2 x halves) after all
            # wave-1 pieces; their SP seqs clear before the first out-group
            # DMA blocks the SP queue.
            def w1_slot(kp):
                if kp == 0:
                    # consume eh0 in col-halves as the chunked prep lands;
                    # start=True zeroes the whole 2KB bank region, so only
                    # the first half carries it
                    h = O_PER // 2
                    for hi_, nsl in enumerate((slice(0, h), slice(h, O_PER))):
                        for m in range(W1):
                            mm(pn[m], eh, 0, m, start=(hi_ == 0), nsl=nsl)
                    return
                if kp == 1:
                    h = O_PER // 2
                    for nsl in (slice(0, h), slice(h, O_PER)):
                        for m in range(W1):
                            mm(pn[m], q1, 0, m, nsl=nsl)
                else:
                    for m in range(W1):
                        mm(pn[m], q1, kp - 1, m)
                if 0 <= kp - 2 < NQ2:
                    for m in range(W1):
                        mm(pn[m], q2, kp - 2, m)
                for m in range(W1):
                    mm(pn[m], eh, kp, m)

            issue_w_and_prep(0)
            issue_w_and_prep(1)
            issue_x(0, "a")
            w1_slot(0)
            for kp in range(1, KP):
                if kp + 1 < KP:
                    issue_w_and_prep(kp + 1)
                issue_x(kp, "a")
                if kp == KP - 1:
                    # warm the Sign table right after the last exp so the
                    # 1283ns table load hides before the first ACT evict
                    nc.scalar.activation(warm[:], warm[:], Act.Sign)
                w1_slot(kp)
            for kp in range(KP):
                issue_x(kp, "b")

            # wave-1 tail: close each m with the last q1 pass (+ any q2
            # passes not yet consumed by the slot stagger), evict on
            # alternating engines, fire the out-group DMAs as they fill
            for m in range(W1):
                for kq in range(NQ2):
                    if kq + 2 >= KP:
                        mm(pn[m], q2, kq, m)
                mm(pn[m], q1, KP - 1, m, stop=True)
                evict(pn[m], m, eng_act=(m % 2 == 1))
                if m == 3:
                    group_dma(0)
            group_dma(1)

            # ---- wave 2: all planes resident ------------------------------
            pn2 = {m: psum_tile(m) for m in range(W1, MT)}
            for kp in range(SWEEP_KP):
                for m in range(W1, MT):
                    mm(pn2[m], eh, kp, m, start=(kp == 0))
                    mm(pn2[m], q1, kp, m)
                    if kp < NQ2:
                        mm(pn2[m], q2, kp, m)
            # m-sweep over the last kps staggers the closes so the evicts
            # pipeline behind the PE instead of bursting at the end
            for m in range(W1, MT):
                for kp in range(SWEEP_KP, KP):
                    mm(pn2[m], eh, kp, m)
                    mm(pn2[m], q1, kp, m, stop=(kp == KP - 1))
                if m < MT - 1:
                    evict(pn2[m], m, eng_act=(m % 2 == 1))
                else:
                    # last m-tile: evict halves on both engines in parallel
                    h = O_PER // 2
                    evict(pn2[m], m, eng_act=False, nsl=slice(0, h))
                    evict(pn2[m], m, eng_act=True, nsl=slice(h, O_PER))
                gi, mi = group_of(m)
                if mi == OGROUPS[gi][1] - OGROUPS[gi][0] - 1:
                    group_dma(gi)
            prep_and_trigger_tail()

    if split_waits:
        _split_multi_waits(nc)
    return nc


def _get_compiled():
    if "k" not in _COMPILED:
        _COMPILED["k"] = _build()
    return _COMPILED["k"]


def _encode_x_fp8(x):
    """x is exactly {0.0, 1.0}; encode 2x-1 in {-1,+1} as fp8e4 bytes
    (+1.0 = 0x38, -1.0 = 0xB8) - a lossless re-encoding of the input."""
    import ml_dtypes

    enc = np.where(np.asarray(x) > 0.5, np.uint8(0x38), np.uint8(0xB8))
    return enc.view(ml_dtypes.float8_e4m3)


def kernel(x: np.ndarray, raw_weight: np.ndarray, _trace: bool = False):
    from concourse.bass_utils import run_bass_kernel_spmd

    nc = _get_compiled()

    x = np.asarray(x)
    raw_weight = np.asarray(raw_weight)

    xT = np.ascontiguousarray(_encode_x_fp8(x).T)  # [IN_F, BATCH] fp8
    wT = np.round(raw_weight.T * 4096.0).astype(np.int16)

    in_maps = []
    for c in range(N_CORES):
        bg, og = divmod(c, OG)
        in_maps.append(
            {
                "xt": np.ascontiguousarray(xT[:, bg * B_PER : (bg + 1) * B_PER]),
                "wt": np.ascontiguousarray(wT[:, og * O_PER : (og + 1) * O_PER]),
            }
        )

    res = run_bass_kernel_spmd(
        nc, in_maps, core_ids=list(range(N_CORES)), trace=_trace
    )

    full = np.empty((BATCH, OUT_F), dtype=x.dtype)
    for c in range(N_CORES):
        bg, og = divmod(c, OG)
        # DVE evicts write {0,1}; ACT Sign evicts write the +-1 code
        # (u8 cast of -1 may be 0 or 255 depending on saturation) - the
        # decode "code == 1" is correct for both.
        blk = np.asarray(res.results[c]["out"])
        full[bg * B_PER : (bg + 1) * B_PER, og * O_PER : (og + 1) * O_PER] = (
            blk == 1
        ).astype(x.dtype)
    if _trace:
        kernel.last_results = res
    return full


# revision 50
# speedup vs baseline: 5.6534x; 5.6534x over previous
"""DigitalMapper kernel for 8 trn2 NeuronCores.

Math: reference computes  out = (x @ softmax(W, axis=1).T) > 0.5  with
x in {0,1}.  Let E = exp(W) (row-unnormalized).  Then

  out[b,o] > 0.5  <=>  sum_i (2*x[b,i] - 1) * E[o,i] > 0

so the softmax divide, the row-max subtraction and the threshold fold
into a zero-test on a +-1 matmul (|W| <= 5.5 so exp(W) <= 185 fits
fp8e4's 240 max).

All-fp8 DoubleRow pipeline (0.5 PE-cycles/contraction-row): E is split
on-device into a 2-term fp8e4 cascade

  Q0 = e4m3(E);  Q1 = e4m3(E - Q0)

giving ~10.3 effective mantissa bits.  Measured 1640/8.4M output
bit-flips vs the f64 reference = rel 1.98e-2, under the 2e-2 gate
(NQ2 extra third-term k-pairs are available as margin fallback).
x is +-1, exact in fp8e4; both passes accumulate into one PSUM bank:

  S = xb @ (Q0 + Q1).T   (fp32 PSUM),  out = S > 0.

Per-kp prep chain (engine-balanced, all rates under the 1.71us/kp PE
slot rate): ACT exp->f32 ef (1.04us); DVE cast ef->fp8 eh (0.59us,
2x_2p mode); Pool sub (ef - eh)->fp8 q1 (0.85us).

Sharding: 2 batch-groups x 4 out-feature groups.  Each core gets
x.T[:, bg*2048:...] host-encoded as fp8e4 +-1 bytes (bijective
re-encoding of the 0/1 input) and W.T[:, og*512:...] as round(W*4096)
int16 (halves the weight DMA; the 2^-13 quantization is far below the
fp8-cascade residual), and produces a [2048, 512] block.

Schedule: wave-1 = m-tiles 0..7 kp-slotted against the input stream
(slot k: q1[k-1] then hi[k] passes), tail closes m0..7 with evictions
alternating DVE is_gt / ACT Sign (Sign table pre-warmed after the last
exp; host maps the +-1 code back to {0,1}).  Wave-2 = m8..15, all
planes resident: kp-major 0..4, then an m-sweep over kp 5..7 that
staggers the closes so evictions pipeline behind the PE instead of
bursting at the end.  Output DMAs are batched [4,4,4,2,1,1] m-tiles
to cut SP descriptor time; the last groups stay small for tail latency.
"""

import sys

sys.path.insert(0, "/opt/trn_rl_repo")

import numpy as np

BATCH, IN_F, OUT_F = 4096, 2048, 2048
N_CORES = 8
BG, OG = 2, 4  # batch groups x out-feature groups
B_PER = BATCH // BG  # 2048 batch rows per core
O_PER = OUT_F // OG  # 512 out features per core
P = 128
KT = IN_F // P  # 16 contraction tiles
KP = KT // 2  # 8 DoubleRow k-pairs
MT = B_PER // P  # 16 output row tiles per core
W1 = 8  # wave-1 m-tiles (= PSUM banks)
NQ2 = 0  # k-pairs carrying a 3rd (q2) residual pass; 0 measures
         # 1640/8.4M flips (rel 1.98e-2) - under the 2e-2 gate.
         # Bump to 1 (rel 1.92e-2) if the hw flip count drifts up.
SWEEP_KP = 5  # wave-2: kp >= SWEEP_KP run m-major with inline evicts

_COMPILED = {}


def _patch_tile_drain():
    """walrus in this container allows only ONE sem-wait per CTRL (Drain/NOP)
    instruction; Tile's kernel-tail drain aggregates one wait per live
    semaphore.  Split the waits across a chain of SP nops."""
    import concourse.mybir as mybir
    import concourse.tile as tile_mod
    from concourse.vector_clock import ScopedClock

    if getattr(tile_mod.TileContext, "_drain_split_patched", False):
        return

    def _drain_and_barrier_split(self, tick_clock, wait_clock):
        nc = self.nc
        drain_inst = nc.sync.drain()
        wait_clock.add_sem_waits(
            drain_inst.ins, ScopedClock({None: tick_clock.global_clock})
        )
        si = drain_inst.ins.sync_info
        waits = list(si.on_wait) if si is not None else []
        if len(waits) > 1:
            si.on_wait.clear()
            si.on_wait.extend(waits[:1])
            for w in waits[1:]:
                nop = nc.sync.nop(nofuse=True)
                if nop.ins.sync_info is None:
                    nop.ins.sync_info = mybir.SyncInfo(on_wait=[], on_update=[])
                nop.ins.sync_info.on_wait.append(w)
        nc.all_engine_barrier()
        assert self.sems is not None
        popped = nc._tile_sem_poison_stack.pop()
        assert popped is self._sem_poison
        nc.clear_and_free_semaphores(list(self.sems.allocated().values()))
        nc.all_engine_barrier()

    tile_mod.TileContext._drain_and_barrier = _drain_and_barrier_split
    tile_mod.TileContext._drain_split_patched = True


def _split_multi_waits(nc):
    """walrus here allows very few sem-waits per instruction.  Hoist extra
    waits onto same-engine NOPs placed immediately before the instruction
    (same blocking point, engine executes in order).  DMA-queue instructions
    keep their waits - their sync runs through the DGE queues."""
    import concourse.mybir as mybir

    n = 0
    for f in nc.m.functions:
        for bb in f.blocks:
            new_insts = []
            for inst in bb.instructions:
                si = inst.sync_info
                if si is not None and si.on_wait and len(si.on_wait) > 1:
                    waits = list(si.on_wait)
                    si.on_wait.clear()
                    si.on_wait.append(waits[0])
                    for w in waits[1:]:
                        n += 1
                        new_insts.append(
                            mybir.InstNoOp(
                                name=f"wsplit-{n}",
                                opcode="NoOp",
                                engine=inst.engine,
                                sync_info=mybir.SyncInfo(on_wait=[w], on_update=[]),
                                bass_nofuse=True,
                            )
                        )
                new_insts.append(inst)
            if n:
                try:
                    bb.instructions[:] = new_insts
                except TypeError:
                    bb.instructions = new_insts
    return n


def _build(split_waits: bool = True):
    """One core's SPMD program (see module docstring)."""
    import concourse.bass as bass
    import concourse.mybir as mybir
    import concourse.tile as tile

    _patch_tile_drain()

    f32 = mybir.dt.float32
    f8 = mybir.dt.float8e4
    u8 = mybir.dt.uint8
    i16 = mybir.dt.int16
    Alu = mybir.AluOpType
    Act = mybir.ActivationFunctionType
    DR = mybir.MatmulPerfMode.DoubleRow

    _build.mm_labels = {}
    # The last three output groups go out via prepared kv_writeback
    # descriptors fired by one trigger_dma - the triggered path skips the
    # HWDGE seq+gen+delay chain (~1.7us off the kernel tail)
    nc = bass.Bass()
    # xt holds host-encoded +-1 in fp8e4 bytes, [IN_F, B_PER]
    xt = nc.dram_tensor("xt", [IN_F, B_PER], f8, kind="ExternalInput")
    # W arrives as round(W*4096) int16: halves the weight DMA; the 2^-13
    # quantization is far below the fp8-cascade residual
    wt = nc.dram_tensor("wt", [IN_F, O_PER], i16, kind="ExternalInput")
    out = nc.dram_tensor("out", [B_PER, O_PER], u8, kind="ExternalOutput")

    # out-DMA m-tile groups: big early groups amortize the 500ns SP
    # descriptor time; the tail groups stay small for end latency
    OGROUPS = [(0, 4), (4, 8), (8, 12), (12, 14), (14, 15), (15, 16)]

    with tile.TileContext(nc) as tc:
        with (
            tc.tile_pool(name="xq", bufs=1) as xq_pool,
            tc.tile_pool(name="wr", bufs=3) as wr_pool,
            tc.tile_pool(name="ef", bufs=3) as ef_pool,
            tc.tile_pool(name="eh", bufs=1) as eh_pool,
            tc.tile_pool(name="q1", bufs=1) as q1_pool,
            tc.tile_pool(name="qx", bufs=1) as qx_pool,
            tc.tile_pool(name="ps", bufs=1, space="PSUM") as ps_pool,
            tc.tile_pool(name="ot", bufs=1) as ot_pool,
        ):
            # touch Exp immediately so the ACT table load overlaps the
            # first input DMAs instead of the first exp
            warm = ef_pool.tile([P, 1], f32, name="warm", tag="warm")
            nc.vector.memset(warm[:], 0.0)
            nc.scalar.activation(warm[:], warm[:], Act.Exp)
            # PE p-state warm-up: the clock reaches 2.4GHz only after 3us of
            # CONTINUOUS PE activity (pe_busy_start resets when PE idles), so
            # a bridge of dummy matmuls must keep PE busy from t~0.6 until
            # the first real matmul (~4.1us) or the early real mms pay 2x.
            wmm = ef_pool.tile([P, 2, P], f8, name="wmm", tag="wmm")
            nc.vector.memset(wmm[:], 0.0)
            wm2 = ef_pool.tile([P, 2, O_PER], f8, name="wm2", tag="wm2")
            nc.vector.memset(wm2[:], 0.0)

            xq, eh, q1, q2 = {}, {}, {}, {}

            def issue_x(kp, part):
                # part "a": batch cols 0:1024 (wave-1 m-tiles), issued from
                # the otherwise-idle Pool queue so the w DMAs own SP;
                # part "b": cols 1024:2048 (wave-2), streamed later on SP
                h = B_PER // 2
                if kp not in xq:
                    xq[kp] = xq_pool.tile(
                        [P, 2, B_PER], f8, name=f"xq{kp}", tag=f"xq{kp}"
                    )
                t = xq[kp]
                src = xt[kp * 2 * P : (kp + 1) * 2 * P, :].rearrange(
                    "(j p) b -> p j b", j=2
                )
                if part == "a":
                    nc.sync.dma_start(t[:, :, :h], src[:, :, :h])
                else:
                    nc.sync.dma_start(t[:, :, h:], src[:, :, h:])

            def issue_w_and_prep(kp):
                """DMA w[kp], then: ACT exp->f32 ef; DVE cast->fp8 eh;
                Pool sub (ef-eh)->fp8 q1.  kp<NQ2 additionally produce a
                second residual plane q2 = e4m3(d1 - q1) via an f32 d1."""
                wr = wr_pool.tile([P, 2, O_PER], i16, name="wr", tag="wr")
                src = wt[kp * 2 * P : (kp + 1) * 2 * P, :].rearrange(
                    "(j p) n -> p j n", j=2
                )
                ehk = eh_pool.tile([P, 2, O_PER], f8, name=f"eh{kp}", tag=f"eh{kp}")
                q1k = q1_pool.tile([P, 2, O_PER], f8, name=f"q1{kp}", tag=f"q1{kp}")
                ef = ef_pool.tile([P, 2, O_PER], f32, name="ef", tag="ef")
                # geometric ramp on the first k-pairs so the first eh/q1
                # planes land ~1us earlier (prep in col-chunks; the DMA
                # stays in >=256-col pieces - smaller pieces pay the
                # sub-512B descriptor penalty and gain nothing on the bus)
                if kp == 0:
                    dma_bounds = [0, 256, O_PER]
                    prep_bounds = [0, 128, 256, 384, O_PER]
                elif kp == 1:
                    dma_bounds = [0, O_PER]
                    prep_bounds = [0, 256, O_PER]
                else:
                    dma_bounds = [0, O_PER]
                    prep_bounds = [0, O_PER]
                for c in range(len(dma_bounds) - 1):
                    sl = slice(dma_bounds[c], dma_bounds[c + 1])
                    nc.sync.dma_start(wr[:, :, sl], src[:, :, sl])
                for c in range(len(prep_bounds) - 1):
                    sl = slice(prep_bounds[c], prep_bounds[c + 1])
                    nc.scalar.activation(
                        ef[:, :, sl], wr[:, :, sl], Act.Exp, scale=2.0**-12
                    )
                    nc.vector.tensor_copy(ehk[:, :, sl], ef[:, :, sl])
                    if kp < NQ2:
                        d1 = qx_pool.tile(
                            [P, 2, O_PER], f32, name=f"d1{kp}", tag=f"d1{kp}"
                        )
                        q2k = qx_pool.tile(
                            [P, 2, O_PER], f8, name=f"q2{kp}", tag=f"q2{kp}"
                        )
                        nc.gpsimd.tensor_tensor(
                            d1[:, :, sl], ef[:, :, sl], ehk[:, :, sl], Alu.subtract
                        )
                        nc.vector.tensor_copy(q1k[:, :, sl], d1[:, :, sl])
                        nc.gpsimd.tensor_tensor(
                            q2k[:, :, sl], d1[:, :, sl], q1k[:, :, sl], Alu.subtract
                        )
                        q2[kp] = q2k
                    else:
                        nc.gpsimd.tensor_tensor(
                            q1k[:, :, sl], ef[:, :, sl], ehk[:, :, sl], Alu.subtract
                        )
                eh[kp] = ehk
                q1[kp] = q1k

            def psum_tile(m):
                return ps_pool.tile(
                    [P, O_PER], f32, name=f"pn_{m % W1}", tag=f"pn_{m % W1}"
                )

            pn = {m: psum_tile(m) for m in range(W1)}

            # ramp the PE clock while the first inputs stream in; the real
            # hi(0) pass overwrites this bank via start=True.  Small mms
            # (on the quickly-memset wmm) start the ramp at ~0.4us and
            # bridge until the big wm2 tile is set; big mms then keep PE
            # busy to ~4.2us so the ramp never resets before real work.
            N_WARM_SMALL, N_WARM_BIG = 16, 8
            for i in range(N_WARM_SMALL):
                nc.tensor.matmul(pn[0][:, :P], wmm[:], wmm[:],
                                 start=(i == 0), stop=(i == N_WARM_SMALL - 1),
                                 perf_mode=DR)
            for i in range(N_WARM_BIG):
                nc.tensor.matmul(pn[1][:], wmm[:], wm2[:], start=(i == 0),
                                 stop=(i == N_WARM_BIG - 1), perf_mode=DR)

            def mm(ps, pass_planes, kp, m, start=False, stop=False,
                   nsl=slice(0, O_PER)):
                label = ('hi' if pass_planes is eh else 'q1' if pass_planes is q1 else 'q2', kp, m)
                r = nc.tensor.matmul(
                    ps[:, nsl],
                    xq[kp][:, :, m * P : (m + 1) * P],
                    pass_planes[kp][:, :, nsl],
                    start=start,
                    stop=stop,
                    perf_mode=DR,
                )
                _build.mm_labels[r.ins.name] = label

            ot_tiles = {}
            for gi, (m0, m1) in enumerate(OGROUPS):
                ot_tiles[gi] = ot_pool.tile(
                    [P, m1 - m0, O_PER], u8, name=f"ot{gi}", tag=f"ot{gi}"
                )


            def group_of(m):
                for gi, (m0, m1) in enumerate(OGROUPS):
                    if m0 <= m < m1:
                        return gi, m - m0
                raise AssertionError(m)

            def evict(ps, m, eng_act, nsl=slice(0, O_PER)):
                # threshold S>0.  DVE: is_gt -> {0,1}.  ACT: Sign -> {-1,0,1}
                # (host maps code 1 -> 1.0, everything else -> 0.0).
                gi, mi = group_of(m)
                dst = ot_tiles[gi][:, mi, nsl]
                if eng_act:
                    nc.scalar.activation(dst, ps[:, nsl], Act.Sign)
                else:
                    nc.vector.tensor_scalar(dst, ps[:, nsl], 0.0, None, Alu.is_gt)

            def group_dma(gi):
                m0, m1 = OGROUPS[gi]
                dst = out[m0 * P : m1 * P, :].rearrange(
                    "(j p) n -> p j n", j=m1 - m0
                )
                nc.sync.dma_start(dst, ot_tiles[gi][:])

            # ---- wave 1: kp-slotted while inputs stream --------------------
            # DMA issue order: w one slot ahead of xa so the exp/cast chain
            # hides under the x transfer; xb (wave-2 x halves) after all
            # wave-1 pieces; their SP seqs clear before the first out-group
            # DMA blocks the SP queue.
            def w1_slot(kp):
                if kp == 0:
                    # consume eh0 in col-halves as the chunked prep lands;
                    # start=True zeroes the whole 2KB bank region, so only
                    # the first half carries it
                    h = O_PER // 2
                    for hi_, nsl in enumerate((slice(0, h), slice(h, O_PER))):
                        for m in range(W1):
                            mm(pn[m], eh, 0, m, start=(hi_ == 0), nsl=nsl)
                    return
                if kp == 1:
                    h = O_PER // 2
                    for nsl in (slice(0, h), slice(h, O_PER)):
                        for m in range(W1):
                            mm(pn[m], q1, 0, m, nsl=nsl)
                else:
                    for m in range(W1):
                        mm(pn[m], q1, kp - 1, m)
                if 0 <= kp - 2 < NQ2:
                    for m in range(W1):
                        mm(pn[m], q2, kp - 2, m)
                for m in range(W1):
                    mm(pn[m], eh, kp, m)

            # xa(kp) is emitted two kp ahead of its prep so the Pool queue
            # interleaves [xa(k+2), sub(k)] - each xa lands well before its
            # slot while the subs still pace one slot ahead of consumption
            # SP order [w0, x0a, w1, x1a, w2, x2a, w3, x3a, w4, w5, x4a,
            # w6, x5a, w7, x6a, x7a]: the first 8 DMAs land on fresh DMAHW
            # queues (fast completion sems); later ones pay ~1.7us extra
            # latency, so the tightest-deadline pieces go first and the
            # w-stream (whose exp/cast chain adds ~2us) pulls ahead of the
            # xa-stream from kp4 on.
            SP_ORDER = ["w0", "x0", "w1", "w2", "x1", "w3", "x2", "w4",
                        "x3", "w5", "x4", "w6", "x5", "w7", "x6", "x7"]
            emitted = 0

            def try_slots():
                nonlocal emitted
                while emitted < KP and emitted in xq and emitted in eh:
                    if emitted == KP - 1:
                        # warm the Sign table right after the last exp so
                        # the 1283ns load hides before the first ACT evict
                        nc.scalar.activation(warm[:], warm[:], Act.Sign)
                    w1_slot(emitted)
                    emitted += 1

            for item in SP_ORDER:
                if item[0] == "w":
                    issue_w_and_prep(int(item[1]))
                else:
                    issue_x(int(item[1]), "a")
                try_slots()
            for kp in range(KP):
                issue_x(kp, "b")

            # wave-1 tail: close each m with the last q1 pass (+ any q2
            # passes not yet consumed by the slot stagger), evict on
            # alternating engines, fire the out-group DMAs as they fill
            for m in range(W1):
                for kq in range(NQ2):
                    if kq + 2 >= KP:
                        mm(pn[m], q2, kq, m)
                mm(pn[m], q1, KP - 1, m, stop=True)
                evict(pn[m], m, eng_act=(m % 2 == 1))
                if m == 3:
                    group_dma(0)
            group_dma(1)

            # ---- wave 2: all planes resident ------------------------------
            pn2 = {m: psum_tile(m) for m in range(W1, MT)}
            for kp in range(SWEEP_KP):
                for m in range(W1, MT):
                    mm(pn2[m], eh, kp, m, start=(kp == 0))
                    mm(pn2[m], q1, kp, m)
                    if kp < NQ2:
                        mm(pn2[m], q2, kp, m)
            # m-sweep over the last kps staggers the closes so the evicts
            # pipeline behind the PE instead of bursting at the end
            for m in range(W1, MT):
                for kp in range(SWEEP_KP, KP):
                    mm(pn2[m], eh, kp, m)
                    mm(pn2[m], q1, kp, m, stop=(kp == KP - 1))
                if m < MT - 1:
                    # m13/m14 -> ACT so DVE is free the moment the last
                    # m-tile closes
                    ea = True if m in (13, 14) else m % 2 == 1
                    evict(pn2[m], m, eng_act=ea)
                else:
                    evict(pn2[m], m, eng_act=False)
                gi, mi = group_of(m)
                if mi == OGROUPS[gi][1] - OGROUPS[gi][0] - 1:
                    group_dma(gi)

    if split_waits:
        _split_multi_waits(nc)
    return nc


def _get_compiled():
    if "k" not in _COMPILED:
        _COMPILED["k"] = _build()
    return _COMPILED["k"]


def _encode_x_fp8(x):
    """x is exactly {0.0, 1.0}; encode 2x-1 in {-1,+1} as fp8e4 bytes
    (+1.0 = 0x38, -1.0 = 0xB8) - a lossless re-encoding of the input."""
    import ml_dtypes

    enc = np.where(np.asarray(x) > 0.5, np.uint8(0x38), np.uint8(0xB8))
    return enc.view(ml_dtypes.float8_e4m3)


def kernel(x: np.ndarray, raw_weight: np.ndarray, _trace: bool = False):
    from concourse.bass_utils import run_bass_kernel_spmd

    nc = _get_compiled()

    x = np.asarray(x)
    raw_weight = np.asarray(raw_weight)

    xT = np.ascontiguousarray(_encode_x_fp8(x).T)  # [IN_F, BATCH] fp8
    wT = np.round(raw_weight.T * 4096.0).astype(np.int16)

    in_maps = []
    for c in range(N_CORES):
        bg, og = divmod(c, OG)
        in_maps.append(
            {
                "xt": np.ascontiguousarray(xT[:, bg * B_PER : (bg + 1) * B_PER]),
                "wt": np.ascontiguousarray(wT[:, og * O_PER : (og + 1) * O_PER]),
            }
        )

    res = run_bass_kernel_spmd(
        nc, in_maps, core_ids=list(range(N_CORES)), trace=_trace
    )

    full = np.empty((BATCH, OUT_F), dtype=x.dtype)
    for c in range(N_CORES):
        bg, og = divmod(c, OG)
        # DVE evicts write {0,1}; ACT Sign evicts write the +-1 code
        # (u8 cast of -1 may be 0 or 255 depending on saturation) - the
        # decode "code == 1" is correct for both.
        blk = np.asarray(res.results[c]["out"])
        full[bg * B_PER : (bg + 1) * B_PER, og * O_PER : (og + 1) * O_PER] = (
            blk == 1
        ).astype(x.dtype)
    if _trace:
        kernel.last_results = res
    return full
